# revision 18
# baseline (speedup 1.0000x reference)
"""Baichuan attention on 8 Trainium2 NeuronCores — tensor-parallel over heads.

Sharding: core c computes heads [4c, 4c+4): its slice of the fused QKV
projection, attention for those heads, then 1/8 of o_proj's output columns
after an AllGather of the per-core context slices (moves 4MB/rank instead of
a 32MB AllReduce of partial sums; mathematically identical to the module's
world_size logic).

Layout: scores are computed transposed (scoresT[k, q] blocks) so the PE
contraction dim always sits on SBUF partitions and every matmul streams a
512-wide moving operand. Matmul operands are fp16 (1 cyc/row on the PE) with
fp32 PSUM accumulation.

Pipeline notes (evolved from per-phase traces; v1 was PE 87% busy with
102us of dependency gaps):
- all inputs are pre-arranged host-side into SBUF-partition-major layouts
  ([128, ...] with each partition's data contiguous in DRAM), so every DMA
  moves >=8KB per descriptor instead of the 256B slivers a column-sliced
  weight fetch generates (those ran ~6us/MB and starved the PE).
- scores land in [128, 2x512] two-bank PSUM tiles so ONE exp activation
  covers a k-tile pair: halves the Scalar engine's per-tile overhead, which
  otherwise paces the PE in the attention phase.
- the causal mask (fp16) is copied into the PSUM banks ahead of each masked
  pair and the scores matmuls run with start=False (accumulate onto it):
  no DVE mask-adds on the critical path.
- softmax denominators use reciprocal_approx_fast (1 DVE op, ~18 good bits)
  instead of reciprocal (4us each); the normalize multiply reads the PSUM
  accumulator directly, so banks recycle ~1us after the last matmul.
- collectives: a tiny warmup AllGather during phase 1 absorbs the one-time
  CC-stream setup; per-head-pair gathers fire as each half-block of heads
  finishes. ALL o_proj runs as a dense tail after the last attention block:
  by then every gather has had >=100us to land, so cross-rank skew in the
  collectives can never block the in-order PE queue. ct tiles are fetched
  from shared DRAM just-in-time during the tail.
- PSUM: scores 2x2 banks + ps_o 1 + ps_row 1 + o_proj 2 = 8.
"""

import numpy as np

import concourse.bacc as bacc
import concourse.mybir as mybir
import concourse.tile as tile
from concourse.bass_utils import run_bass_kernel_spmd

F32 = mybir.dt.float32

N_CORES = 8
NUM_HEADS = 32
HEAD_DIM = 128
P = 128          # SBUF partitions / PE contraction tile
SQ = 512         # s_q block width (PSUM bank = 512 fp32)
MM_MODE = "f16"  # 'f16' | 'f32' (operand dtype for matmuls)

_CACHE: dict = {}


def _mm_dtype(mode):
    return {"f16": mybir.dt.float16, "f32": F32}[mode]


def _mask_pair_list(block_cls):
    """(t0, b) for every masked k-tile pair, in consumption order."""
    return sorted({(t - t % 2, b) for (t, b), v in block_cls.items()
                   if v == "mask"}, key=lambda k: (k[1], k[0]))


def build(S, H, block_cls, mode=MM_MODE):
    """Build the SPMD program. block_cls[(t, b)] = 'plain' | 'mask' for every
    computed scoresT block ([128 s_k] x [SQ s_q]); absent = fully masked, skip.
    """
    MD = _mm_dtype(mode)
    hpc = NUM_HEADS // N_CORES          # heads per core
    dpc = hpc * HEAD_DIM                # per-core slice of the hidden dim
    n_ht = H // P                       # contraction tiles for QKV/o_proj
    n_qk = 2 * dpc // P                 # q+k output tiles
    n_sq = S // SQ                      # s_q blocks
    n_st = S // P                       # s_k tiles
    scale = 1.0 / np.sqrt(np.float32(HEAD_DIM))
    s_half = S // 2
    sb_per_half = s_half // SQ
    n_hh = n_ht // 2                    # o_proj k-tiles per gather half
    mask_pairs = _mask_pair_list(block_cls)
    mp_slot = {k: i for i, k in enumerate(mask_pairs)}

    nc = bacc.Bacc("TRN2", target_bir_lowering=False, debug=False,
                   num_devices=N_CORES)

    # all inputs pre-arranged host-side: [128 partitions, ...] with each
    # partition's bytes contiguous in DRAM (big DMA descriptors)
    x_d = nc.dram_tensor("x_sb", [P, S // SQ, n_ht, SQ], MD,
                         kind="ExternalInput")
    wqk_d = nc.dram_tensor("wqk_sb", [P, n_qk, n_ht, P], MD,
                           kind="ExternalInput")
    wv_d = nc.dram_tensor("wv_sb", [P, n_ht, dpc], MD, kind="ExternalInput")
    wo_d = nc.dram_tensor("wo_sb", [P, n_ht, dpc], MD, kind="ExternalInput")
    mask_d = nc.dram_tensor("mask_sb", [P, max(len(mask_pairs), 1), 2, SQ],
                            MD, kind="ExternalInput")
    out_cols = nc.dram_tensor("out_cols", [S, dpc], F32, kind="ExternalOutput")

    # AllGather in head-pair chunks: gat[b][pp] holds local heads
    # {2pp, 2pp+1} for s_q block b; ct[b][pp] gathers those pairs from all
    # ranks. o_proj consumes them against host-permuted w_o rows.
    gat_b = [[nc.dram_tensor(f"gat_{b}_{pp}", [dpc // 2, SQ], MD)
              for pp in range(2)] for b in range(n_sq)]
    ct_b = [[nc.dram_tensor(f"ct_{b}_{pp}", [H // 2, SQ], MD,
                            addr_space="Shared") for pp in range(2)]
            for b in range(n_sq)]
    # tiny warmup collective: absorbs the one-time CC-stream/HAM setup
    # during phase 1 so the first real AllGather runs at steady latency
    warm_in = nc.dram_tensor("warm_in", [1, 64], MD)
    warm_out = nc.dram_tensor("warm_out", [1, 64 * N_CORES], MD,
                              addr_space="Shared")

    with tile.TileContext(nc) as tc:
        with (
            tc.tile_pool(name="consts", bufs=1) as cpool,
            tc.tile_pool(name="span", bufs=1) as span,
        ):
            ones_f = cpool.tile([P, P], F32, tag="ones_f")
            nc.gpsimd.memset(ones_f[:], 1.0)
            ones_sq = cpool.tile([P, P], MD, tag="ones_sq")
            nc.scalar.copy(ones_sq[:], ones_f[:])
            nc.gpsimd.collective_compute(
                "AllGather", mybir.AluOpType.bypass,
                replica_groups=[list(range(N_CORES))],
                ins=[warm_in.ap().opt()], outs=[warm_out.ap().opt()])

            # v ([s_k, d] natural, all heads) and q/k (transposed, all heads)
            # live in SBUF across phases 1-2; QKV evictions write them
            # directly (no DRAM bounce)
            v_sb = span.tile([P, n_st, dpc], MD, tag="v")
            qk_all = span.tile([P, n_qk, S], MD, tag="qk")

            # =============== phase 1: QKV projection ===============
            # sb-outer: all 8 q/k output chains run against one resident x
            # quarter, so the next quarter has a full 66us to arrive.
            # Queues: sync feeds w-tile 0 + x quarter 0 (chunked so the PE
            # starts ~3us in), scalar feeds the odd quarters, gpsimd streams
            # the weight tiles (re-DMA'd per quarter: cheap with the big-
            # descriptor layout, and a bufs=3 ring beats 8 resident tiles).
            with (
                tc.tile_pool(name="qkv_x", bufs=1) as xpool,
                tc.tile_pool(name="qkv_w", bufs=3) as wpool,
                tc.tile_pool(name="qkv_wv", bufs=1) as wvpool,
                tc.tile_pool(name="qkv_ps", bufs=4, space="PSUM") as pspool,
            ):
                wv_sb = wvpool.tile([P, n_ht, dpc], MD, tag="wv")
                for half in range(2):
                    xq = []
                    for sb in range(sb_per_half):
                        q_idx = half * sb_per_half + sb
                        x_tile = xpool.tile([P, n_ht, SQ], MD, tag="x",
                                            bufs=3, name="x_tile")
                        if q_idx == 0:
                            for c in range(4):
                                nc.sync.dma_start(
                                    x_tile[:, 8 * c:8 * (c + 1), :],
                                    x_d.ap()[:, 0, 8 * c:8 * (c + 1), :])
                        else:
                            eng = nc.sync if sb % 2 == 0 else nc.scalar
                            eng.dma_start(x_tile[:], x_d.ap()[:, q_idx, :, :])
                        xq.append(x_tile)
                    for sb in range(sb_per_half):
                        lo = half * s_half + sb * SQ
                        for ot in range(n_qk):
                            w_tile = wpool.tile([P, n_ht, P], MD, tag="w",
                                                name="w_tile")
                            if half == 0 and sb == 0 and ot == 0:
                                # ahead of the x stream on the sync queue
                                for c in range(2):
                                    nc.sync.dma_start(
                                        w_tile[:, 16 * c:16 * (c + 1), :],
                                        wqk_d.ap()[:, 0,
                                                   16 * c:16 * (c + 1), :])
                            else:
                                nc.gpsimd.dma_start(
                                    w_tile[:], wqk_d.ap()[:, ot, :, :])
                            ps = pspool.tile([P, SQ], F32, tag="qkv")
                            for t in range(n_ht):
                                nc.tensor.matmul(
                                    ps[:],
                                    w_tile[:, t, :],
                                    xq[sb][:, t, :],
                                    start=(t == 0), stop=(t == n_ht - 1))
                            # fold the softmax scale into q at eviction;
                            # write straight into the resident qk tile
                            mul = scale if ot < dpc // P else 1.0
                            nc.scalar.mul(qk_all[:, ot, lo:lo + SQ],
                                          ps[:], mul)
                        if half == 0 and sb == 0:
                            nc.gpsimd.dma_start(wv_sb[:], wv_d.ap()[:])
                    # v: psum [s=128, dpc] accumulated over h-tiles
                    for sti in range(s_half // P):
                        st_g = half * (s_half // P) + sti
                        sb, off = (sti * P) // SQ, (sti * P) % SQ
                        ps_v = pspool.tile([P, dpc], F32, tag="qkv")
                        for t in range(n_ht):
                            nc.tensor.matmul(
                                ps_v[:],
                                xq[sb][:, t, off:off + P],
                                wv_sb[:, t, :],
                                start=(t == 0), stop=(t == n_ht - 1))
                        nc.vector.tensor_copy(v_sb[:, st_g, :], ps_v[:])

            # ====== phases 2-4: attention / chunked AllGather / o_proj ======
            with (
                tc.tile_pool(name="at_mask", bufs=1) as mpool,
                tc.tile_pool(name="at_exp", bufs=4) as epool,
                tc.tile_pool(name="at_out", bufs=3) as opool,
                tc.tile_pool(name="at_r", bufs=2) as rpool,
                tc.tile_pool(name="op_w", bufs=1) as owpool,
                tc.tile_pool(name="op_ct", bufs=48) as ctpool,
                tc.tile_pool(name="op_part", bufs=5) as partpool,
                tc.tile_pool(name="op_stage", bufs=3) as ospool,
                tc.tile_pool(name="at_ps_s", bufs=2, space="PSUM") as aps_s,
                tc.tile_pool(name="at_ps_o", bufs=1, space="PSUM") as aps_o,
                tc.tile_pool(name="at_ps_r", bufs=1, space="PSUM") as aps_r,
                tc.tile_pool(name="op_ps", bufs=2, space="PSUM") as opspool,
            ):
                wo_sb = owpool.tile([P, n_ht, dpc], MD, tag="wo")
                # causal-mask pair blocks resident in SBUF (fp16); copied
                # into the PSUM banks ahead of each masked scores pair
                mt_all = None
                if mask_pairs:
                    mt_all = mpool.tile([P, len(mask_pairs), 2, SQ], MD,
                                        tag="mask")
                    nc.gpsimd.dma_start(mt_all[:], mask_d.ap()[:])

                ct_tiles = {}   # (b, pp) -> list of 16 SBUF k-tile views
                partials = {}   # st -> SBUF partial o_proj accumulator

                def prefetch_ct(b, pp):
                    ct_t = ct_b[b][pp].ap().rearrange(
                        "(t p) s -> p t s", p=P)
                    tiles = []
                    for t in range(n_hh):
                        c_t = ctpool.tile([P, SQ], MD, tag="ct")
                        nc.sync.dma_start(c_t[:], ct_t[:, t, :])
                        tiles.append(c_t)
                    ct_tiles[(b, pp)] = tiles

                def emit_oproj_half(bprev, st, pp):
                    cts = ct_tiles[(bprev, pp)]
                    ps = opspool.tile([P, dpc], F32, tag="op", name="op_ps")
                    for tt in range(n_hh):
                        nc.tensor.matmul(
                            ps[:],
                            cts[tt][:, st * P:(st + 1) * P],
                            wo_sb[:, pp * n_hh + tt, :],
                            start=(tt == 0), stop=(tt == n_hh - 1))
                    if pp == 0:
                        part = partpool.tile([P, dpc], F32, tag="part",
                                             name="part")
                        nc.vector.tensor_copy(part[:], ps[:])
                        partials[st] = part
                    else:
                        ob = ospool.tile([P, dpc], F32, tag="ostage",
                                         name="ostage")
                        nc.vector.tensor_add(ob[:], partials[st][:], ps[:])
                        nc.scalar.dma_start(
                            out_cols.ap()[bprev * SQ + st * P:
                                          bprev * SQ + (st + 1) * P, :],
                            ob[:])

                def head_attention(h, b, ts_here):
                    q_sl = qk_all[:, h, b * SQ:(b + 1) * SQ]
                    # ps_o/ps_row allocated lazily AFTER the first scores
                    # pair so the pair tiles claim the PSUM banks the QKV
                    # phase never used (avoids a transition stall)
                    ps_o = ps_row = None
                    npair = len(ts_here) // 2
                    last_pair = npair - 1

                    def emit_evrow(pi, ex, t0, t1):
                        nonlocal ps_o, ps_row
                        if ps_o is None:
                            ps_o = aps_o.tile([P, SQ], F32, tag="out",
                                              name="ps_o")
                            ps_row = aps_r.tile([P, SQ], F32, tag="row",
                                                name="ps_row")
                        for j, t in ((0, t0), (1, t1)):
                            first = pi == 0 and j == 0
                            last = pi == last_pair and j == 1
                            nc.tensor.matmul(
                                ps_o[:], v_sb[:, t, h * P:(h + 1) * P],
                                ex[:, j, :], start=first, stop=last)
                            nc.tensor.matmul(
                                ps_row[:], ones_sq[:], ex[:, j, :],
                                start=first, stop=last)

                    pend = None
                    for pi in range(npair):
                        t0, t1 = ts_here[2 * pi], ts_here[2 * pi + 1]
                        ps_s = aps_s.tile([P, 2, SQ], F32, tag="scores",
                                          name="ps_s")
                        masked = block_cls[(t0, b)] == "mask"
                        assert masked == (block_cls[(t1, b)] == "mask")
                        if masked:
                            # preload the additive mask into both PSUM
                            # banks; the scores matmuls accumulate onto it.
                            # block 0 uses the scalar engine: the DVE queue
                            # is still draining phase-1 v evictions there.
                            src = mt_all[:, mp_slot[(t0, b)], :, :]
                            if b == 0:
                                nc.scalar.copy(ps_s[:], src)
                            else:
                                nc.vector.tensor_copy(ps_s[:], src)
                        for j, t in ((0, t0), (1, t1)):
                            nc.tensor.matmul(
                                ps_s[:, j, :],
                                qk_all[:, hpc + h, t * P:(t + 1) * P],
                                q_sl, start=not masked, stop=True,
                                skip_group_check=masked)
                        ex = epool.tile([P, 2, SQ], MD, tag="exp",
                                        name="ex")
                        nc.scalar.activation(
                            ex[:], ps_s[:],
                            mybir.ActivationFunctionType.Exp)
                        if pend is not None:
                            emit_evrow(*pend)
                        pend = (pi, ex, t0, t1)
                    emit_evrow(*pend)

                    recip = rpool.tile([P, SQ], F32, tag="recip",
                                       name="recip")
                    nc.vector.reciprocal_approx_fast(recip[:], ps_row[:])
                    ob = opool.tile([P, SQ], MD, tag="ob", name="ob")
                    nc.vector.tensor_mul(ob[:], ps_o[:], recip[:])
                    nc.scalar.dma_start(
                        gat_b[b][h // 2].ap()[(h % 2) * P:
                                              (h % 2 + 1) * P, :], ob[:])

                for b in range(n_sq):
                    ts_here = [t for t in range(n_st) if (t, b) in block_cls]
                    assert ts_here and len(ts_here) % 2 == 0
                    for h in range(hpc):
                        head_attention(h, b, ts_here)
                        if h % 2 == 1:
                            pp = h // 2
                            nc.gpsimd.collective_compute(
                                "AllGather", mybir.AluOpType.bypass,
                                replica_groups=[list(range(N_CORES))],
                                ins=[gat_b[b][pp].ap().opt()],
                                outs=[ct_b[b][pp].ap().opt()])
                        if b == 0 and h == 0:
                            # wo arrives while block 0's attention runs,
                            # off the startup critical path
                            nc.gpsimd.dma_start(wo_sb[:], wo_d.ap()[:])

                # dense o_proj tail: every AllGather has had >=100us to
                # land, so no chain can block the in-order PE queue on a
                # straggler rank. ct comes out of shared DRAM just-in-time.
                prefetch_ct(0, 0)
                prefetch_ct(0, 1)
                for b in range(n_sq):
                    for ch in range(8):
                        st, pp = ch % 4, ch // 4
                        emit_oproj_half(b, st, pp)
                        if ch == 3 and b + 1 < n_sq:
                            prefetch_ct(b + 1, 0)
                        if ch == 7 and b + 1 < n_sq:
                            prefetch_ct(b + 1, 1)

    nc.compile()
    return nc


def _classify_blocks(maskT_np, S):
    """Classify each [128, SQ] scoresT block of the (transposed) mask."""
    cls = {}
    for t in range(S // P):
        rows = maskT_np[t * P:(t + 1) * P]
        for b in range(S // SQ):
            blk = rows[:, b * SQ:(b + 1) * SQ]
            if np.all(blk <= -1e30):
                continue                      # fully masked: skip compute
            if np.all(blk == 0.0):
                cls[(t, b)] = "plain"
            else:
                cls[(t, b)] = "mask"
    return cls


def make_in_maps(hidden_states, attention_mask, w_pack, w_o):
    B, S, H = hidden_states.shape
    hpc = NUM_HEADS // N_CORES
    dpc = hpc * HEAD_DIM
    n_ht = H // P
    n_qk = 2 * dpc // P
    np_md = mybir.dt.np(_mm_dtype(MM_MODE))

    def part_major(aT, blocks):
        """[H, W] -> [128, blocks, n_ht, W/blocks]: partition-major with
        each partition's data contiguous (one big DMA descriptor each)."""
        Hh, W = aT.shape
        c = W // blocks
        return np.ascontiguousarray(
            aT.reshape(n_ht, P, blocks, c).transpose(1, 2, 0, 3))

    xT = hidden_states[0].T.astype(np_md)                      # [H, S]
    maskT_np = np.ascontiguousarray(
        np.broadcast_to(attention_mask, (1, 1, S, S))[0, 0].T,
        dtype=np.float32)
    # mask ships as fp16 (clipped): anything under ~-100 zeroes the exp,
    # and halving the bytes halves the PSUM preload cost
    mask16 = np.clip(maskT_np, np.finfo(np.float16).min,
                     np.finfo(np.float16).max).astype(np.float16)
    block_cls = _classify_blocks(maskT_np, S)
    mask_pairs = _mask_pair_list(block_cls)
    # [128, n_pairs, 2, SQ] partition-major mask pair blocks
    mask_sb = np.zeros((P, max(len(mask_pairs), 1), 2, SQ), dtype=np.float16)
    for i, (t0, b) in enumerate(mask_pairs):
        for j in range(2):
            mask_sb[:, i, j, :] = mask16[(t0 + j) * P:(t0 + j + 1) * P,
                                         b * SQ:(b + 1) * SQ]

    # w_o rows permuted to match the head-pair AllGather layout:
    # [pp][rank][head-in-pair] blocks of 128
    perm = np.concatenate(
        [np.arange(128 * (4 * r + 2 * pp + hh),
                   128 * (4 * r + 2 * pp + hh) + 128)
         for pp in (0, 1) for r in range(N_CORES) for hh in (0, 1)])
    x_sb = part_major(xT, S // SQ)
    in_maps = []
    for c in range(N_CORES):
        sl = slice(c * dpc, (c + 1) * dpc)
        wqk_c = np.concatenate(
            [w_pack[0 * H:1 * H][sl], w_pack[1 * H:2 * H][sl]], axis=0)
        woT_c = np.ascontiguousarray(w_o[sl].T)[perm]
        in_maps.append({
            "x_sb": x_sb,
            "wqk_sb": part_major(wqk_c.T.astype(np_md), n_qk),
            "wv_sb": part_major(w_pack[2 * H:3 * H][sl].T.astype(np_md), 1
                                ).reshape(P, n_ht, dpc),
            "mask_sb": mask_sb,
            "wo_sb": part_major(woT_c.astype(np_md), 1).reshape(P, n_ht,
                                                               dpc),
        })
    return in_maps, maskT_np


def kernel(hidden_states, attention_mask, w_pack, w_o):
    B, S, H = hidden_states.shape
    assert B == 1 and H == NUM_HEADS * HEAD_DIM
    assert S % (2 * SQ) == 0

    in_maps, maskT_np = make_in_maps(hidden_states, attention_mask,
                                     w_pack, w_o)
    block_cls = _classify_blocks(maskT_np, S)

    key = (S, H, tuple(sorted(block_cls.items())), MM_MODE)
    if key not in _CACHE:
        _CACHE[key] = build(S, H, block_cls, MM_MODE)
    nc = _CACHE[key]

    res = run_bass_kernel_spmd(nc, in_maps, core_ids=list(range(N_CORES)))
    out = np.concatenate(
        [res.results[c]["out_cols"] for c in range(N_CORES)], axis=1)
    return out.reshape(1, S, H).astype(np.float32)
